# revision 1
# baseline (speedup 1.0000x reference)
"""BikeFlowGNN Trainium2 kernel (self-contained).

kernel(**inputs) takes the FULL unsharded inputs (as produced by the
problem's setup_inputs) and returns the FULL [100000] output, running a
Bass/Tile SPMD program on 8 NeuronCores.

Sharding: nodes partitioned contiguously across the 8 cores (graph
parallel). Per GCN layer each core:
  - holds a replicated fp16 table T of u = dinv*h (node-major, per-rank
    row padding so every shard is window-aligned)
  - dma_gathers its edges' source rows (edges dst-sorted into 128-dst
    windows; int16 low/high index split at table row 32768)
  - aggregates messages with one-hot matmuls accumulating in PSUM
    (feature-major output), applies dinv, multiplies by W (fp32r),
    adds bias, BatchNorm with AllReduce'd stats, relu, rescales by dinv
  - PE-transposes its shard to node-major and AllGathers into T
The pair MLP shards the 100k prediction pairs (4 buckets by src/tgt
index half), processed in SBUF-sized chunks.
"""

import dataclasses
import numpy as np

import concourse.bacc as bacc
import concourse.tile as tile
import concourse.mybir as mybir

F32 = mybir.dt.float32
F32R = mybir.dt.float32r
F16 = mybir.dt.float16
I16 = mybir.dt.int16
I32 = mybir.dt.int32
AF = mybir.ActivationFunctionType
ALU = mybir.AluOpType


@dataclasses.dataclass(frozen=True)
class Cfg:
    n: int = 50000
    e: int = 1600000
    p: int = 100000
    f_in: int = 32
    h: int = 128
    tdim: int = 2
    L: int = 3
    eps: float = 1e-5
    ncores: int = 8
    half: int = 32768
    win: int = 128
    h2: int = 64
    pchunk: int = 2048
    # static padded sizes (from host prep)
    s_low: int = 0
    s_high: int = 0
    bsz: tuple = ()

    @property
    def s_nodes(self):
        return self.n // self.ncores

    @property
    def nwin(self):
        return (self.s_nodes + self.win - 1) // self.win

    @property
    def spad(self):
        return self.nwin * self.win

    @property
    def tpad(self):
        return self.spad * self.ncores

    @property
    def C(self):
        return (self.s_low + self.s_high) // 128

    @property
    def CL(self):
        return self.s_low // 128

    @property
    def bp(self):
        return sum(self.bsz)


# ---------------------------------------------------------------------------
# Host preprocessing (index manipulation only)
# ---------------------------------------------------------------------------

def _wrap16(a):
    """[..., k] -> [..., 16, k/16] with element i at [i%16, i//16]."""
    assert a.shape[-1] % 16 == 0
    return np.ascontiguousarray(
        a.reshape(a.shape[:-1] + (a.shape[-1] // 16, 16)).swapaxes(-1, -2))


def host_prep(cfg0: Cfg, edge_index, source_nodes, target_nodes):
    S = cfg0.s_nodes
    NWIN, WIN, NC = cfg0.nwin, cfg0.win, cfg0.ncores
    SPAD, HALF = cfg0.spad, cfg0.half

    src = np.asarray(edge_index[0], np.int64)
    dst = np.asarray(edge_index[1], np.int64)

    # Degree-balanced node relabeling: deal nodes (sorted by in-degree)
    # round-robin across the (core, window) slots so every window sees a
    # near-equal edge count -> minimal static gather padding.
    deg_in = np.bincount(dst, minlength=cfg0.n)
    order_nodes = np.argsort(-deg_in, kind="stable")
    caps = np.full(NC * NWIN, WIN, np.int64)
    caps[NWIN - 1::NWIN] = S - (NWIN - 1) * WIN
    perm = np.empty(cfg0.n, np.int64)
    taken = 0
    for rnd in range(int(caps.max())):
        live = np.nonzero(caps > rnd)[0]
        k = min(len(live), cfg0.n - taken)
        nodes = order_nodes[taken:taken + k]
        taken += k
        lv = live[:k]
        perm[nodes] = (lv // NWIN) * S + (lv % NWIN) * WIN + rnd
        if taken >= cfg0.n:
            break
    assert taken == cfg0.n
    src = perm[src]
    dst = perm[dst]

    trow = (src // S) * SPAD + (src % S)
    lo = trow < HALF
    core = dst // S
    win = (dst % S) // WIN
    dloc = (dst % S) - win * WIN

    grp = (core * NWIN + win) * 2 + (1 - lo.astype(np.int64))
    ngrp = NC * NWIN * 2
    cnt = np.bincount(grp, minlength=ngrp).reshape(NC, NWIN, 2)
    s_low = max(128, int(np.ceil(cnt[:, :, 0].max() / 128) * 128))
    s_high = max(128, int(np.ceil(cnt[:, :, 1].max() / 128) * 128))
    C = (s_low + s_high) // 128

    order = np.argsort(grp, kind="stable")
    grp_s = grp[order]
    starts = np.zeros(ngrp + 1, np.int64)
    np.cumsum(np.bincount(grp_s, minlength=ngrp), out=starts[1:])
    within = np.arange(len(grp_s)) - starts[grp_s]

    glo = grp_s % 2 == 0
    gc = grp_s // (2 * NWIN)
    gw = (grp_s // 2) % NWIN

    idx_low = np.zeros((NC, NWIN, s_low), np.int16)
    idx_high = np.zeros((NC, NWIN, s_high), np.int16)
    dstloc = np.full((NC, NWIN, C * 128), -1.0, np.float16)

    tr_s, dl_s = trow[order], dloc[order]
    m = glo
    idx_low[gc[m], gw[m], within[m]] = tr_s[m].astype(np.int16)
    dstloc[gc[m], gw[m], within[m]] = dl_s[m].astype(np.float16)
    m = ~glo
    idx_high[gc[m], gw[m], within[m]] = (tr_s[m] - HALF).astype(np.int16)
    dstloc[gc[m], gw[m], s_low + within[m]] = dl_s[m].astype(np.float16)

    idxlo_img = np.tile(_wrap16(idx_low).transpose(0, 2, 1, 3)
                        .reshape(NC, 16, NWIN * s_low // 16), (1, 8, 1))
    idxhi_img = np.tile(_wrap16(idx_high).transpose(0, 2, 1, 3)
                        .reshape(NC, 16, NWIN * s_high // 16), (1, 8, 1))
    dst_img = dstloc.reshape(NC, NWIN, C, 128).transpose(0, 3, 1, 2) \
                    .reshape(NC, 128, NWIN * C)

    # ---- pairs: 4 buckets by (src-half, tgt-half)
    PC = cfg0.p // NC
    sn = perm[np.asarray(source_nodes, np.int64)].reshape(NC, PC)
    tn = perm[np.asarray(target_nodes, np.int64)].reshape(NC, PC)
    srow = (sn // S) * SPAD + (sn % S)
    trow_p = (tn // S) * SPAD + (tn % S)
    bucket = (srow >= HALF).astype(np.int64) * 2 + (trow_p >= HALF).astype(np.int64)
    bsz = []
    for b in range(4):
        bsz.append(max(128, int(np.ceil((bucket == b).sum(1).max() / 128) * 128)))
    BP = sum(bsz)
    boff = np.concatenate([[0], np.cumsum(bsz)])

    pidx = np.zeros((NC, 2, BP), np.int16)
    ppos = np.zeros((NC, PC), np.int64)
    for c in range(NC):
        o = np.argsort(bucket[c], kind="stable")
        bc = bucket[c][o]
        within = np.arange(PC) - np.searchsorted(bc, bc)
        pos = boff[bc] + within
        sr, tr = srow[c][o], trow_p[c][o]
        pidx[c, 0, pos] = np.where(sr < HALF, sr, sr - HALF).astype(np.int16)
        pidx[c, 1, pos] = np.where(tr < HALF, tr, tr - HALF).astype(np.int16)
        ppos[c, o] = pos
    pidx_img = np.stack([
        np.tile(np.concatenate([_wrap16(pidx[c, 0]), _wrap16(pidx[c, 1])],
                               axis=1), (8, 1))
        for c in range(NC)])

    cfg = dataclasses.replace(cfg0, s_low=s_low, s_high=s_high, bsz=tuple(bsz))
    meta = dict(idxlo=idxlo_img, idxhi=idxhi_img, dstloc=dst_img,
                pidx=pidx_img, ppos=ppos, boff=boff, perm=perm)
    return cfg, meta


def host_inputs(cfg: Cfg, meta, inputs):
    NC, S = cfg.ncores, cfg.s_nodes
    perm = meta["perm"]
    dst = perm[np.asarray(inputs["edge_index"][1], np.int64)]
    deg = (np.bincount(dst, minlength=cfg.n) + 1).astype(np.int32)
    degp = np.ones((NC, cfg.spad), np.int32)
    degp[:, :S] = deg.reshape(NC, S)
    deg_img = degp.reshape(NC, cfg.nwin, 128).transpose(0, 2, 1)  # [NC,128,NWIN]

    iota = np.tile(np.arange(128, dtype=np.float16)[None, :], (128, cfg.C))
    W1 = np.asarray(inputs["W1"], np.float32)
    PC = cfg.p // NC
    tf = np.asarray(inputs["time_feats"], np.float32).reshape(NC, PC, cfg.tdim)

    common = dict(
        iota=iota,
        ident16=np.eye(128, dtype=np.float16),
        ident32=np.eye(128, dtype=np.float32),
        W_emb=np.asarray(inputs["W_emb"], np.float32),
        Ws=np.asarray(inputs["Ws"], np.float32),
        bemb_t=np.asarray(inputs["b_emb"], np.float32).reshape(cfg.h, 1),
        bs_t=np.ascontiguousarray(np.asarray(inputs["bs"], np.float32).T),
        g_t=np.ascontiguousarray(np.asarray(inputs["gammas"], np.float32).T),
        be_t=np.ascontiguousarray(np.asarray(inputs["betas"], np.float32).T),
        W1a=np.ascontiguousarray(W1[:cfg.h]),
        W1b=np.ascontiguousarray(W1[cfg.h:2 * cfg.h]),
        W1c=np.ascontiguousarray(W1[2 * cfg.h:]),
        b1_t=np.asarray(inputs["b1"], np.float32).reshape(cfg.h, 1),
        W2=np.asarray(inputs["W2"], np.float32),
        b2_t=np.asarray(inputs["b2"], np.float32).reshape(cfg.h2, 1),
        W3=np.asarray(inputs["W3"], np.float32),
        b3_t=np.asarray(inputs["b3"], np.float32).reshape(1, 1),
    )

    x_old = np.asarray(inputs["x"], np.float32)
    x = np.empty_like(x_old)
    x[perm] = x_old
    in_maps = []
    for c in range(NC):
        tfe = np.zeros((cfg.tdim, cfg.bp), np.float32)
        tfe[:, meta["ppos"][c]] = tf[c].T
        m = dict(common)
        m.update(
            xs=np.ascontiguousarray(x[c * S:(c + 1) * S]),
            deg=np.ascontiguousarray(deg_img[c]),
            idxlo=meta["idxlo"][c], idxhi=meta["idxhi"][c],
            dstloc=meta["dstloc"][c], pidx=meta["pidx"][c],
            tfe=tfe,
        )
        in_maps.append(m)
    return in_maps


def assemble_output(cfg: Cfg, meta, results):
    NC, PC = cfg.ncores, cfg.p // cfg.ncores
    y = np.zeros(cfg.p, np.float32)
    for c in range(NC):
        y[c * PC:(c + 1) * PC] = results[c]["yout"][meta["ppos"][c]]
    return y


# ---------------------------------------------------------------------------
# Bass program
# ---------------------------------------------------------------------------

def build_program(cfg: Cfg, debug=False):
    NC = cfg.ncores
    H, H2, S, SPAD, NWIN, C, CL = (cfg.h, cfg.h2, cfg.s_nodes, cfg.spad,
                                   cfg.nwin, cfg.C, cfg.CL)
    SL, SH, HALF = cfg.s_low, cfg.s_high, cfg.half
    BP = cfg.bp
    RG = [list(range(NC))]

    nc = bacc.Bacc("TRN2", target_bir_lowering=False, debug=debug,
                   num_devices=NC)

    def param(name, shape, dt_):
        return nc.dram_tensor(name, list(shape), dt_, kind="ExternalInput")

    xs = param("xs", (S, cfg.f_in), F32)
    deg = param("deg", (128, NWIN), I32)
    idxlo = param("idxlo", (128, NWIN * SL // 16), I16)
    idxhi = param("idxhi", (128, NWIN * SH // 16), I16)
    dstloc = param("dstloc", (128, NWIN * C), F16)
    iota = param("iota", (128, C * 128), F16)
    ident16 = param("ident16", (128, 128), F16)
    ident32 = param("ident32", (128, 128), F32)
    W_emb = param("W_emb", (cfg.f_in, H), F32)
    Ws = param("Ws", (cfg.L, H, H), F32)
    bemb_t = param("bemb_t", (H, 1), F32)
    bs_t = param("bs_t", (H, cfg.L), F32)
    g_t = param("g_t", (H, cfg.L), F32)
    be_t = param("be_t", (H, cfg.L), F32)
    W1a = param("W1a", (H, H), F32)
    W1b = param("W1b", (H, H), F32)
    W1c = param("W1c", (cfg.tdim, H), F32)
    b1_t = param("b1_t", (H, 1), F32)
    W2 = param("W2", (H, H2), F32)
    b2_t = param("b2_t", (H2, 1), F32)
    W3 = param("W3", (H2, 1), F32)
    b3_t = param("b3_t", (1, 1), F32)
    pidx = param("pidx", (128, 2 * BP // 16), I16)
    tfe = param("tfe", (cfg.tdim, BP), F32)

    yout = nc.dram_tensor("yout", [BP], F32, kind="ExternalOutput")

    T = nc.dram_tensor("Tbl", [cfg.tpad, H], F16, addr_space="Shared")
    agin = nc.dram_tensor("agin", [SPAD, H], F16)
    stin = nc.dram_tensor("stin", [H, 2], F32)
    stout = nc.dram_tensor("stout", [H, 2], F32, addr_space="Shared")

    with tile.TileContext(nc) as tc, \
         tc.tile_pool(name="const", bufs=1) as cp, \
         tc.tile_pool(name="sm", bufs=2) as smp, \
         tc.tile_pool(name="paggr", bufs=3, space="PSUM") as paggr, \
         tc.tile_pool(name="ptr", bufs=2, space="PSUM") as ptr, \
         tc.tile_pool(name="pz", bufs=2, space="PSUM") as pz:

        def load(p, shape, dt_, src, tag):
            t = p.tile(list(shape), dt_, tag=tag)
            nc.sync.dma_start(out=t[:], in_=src)
            return t

        iota_t = load(cp, (128, C * 128), F16, iota.ap(), "iota")
        id16 = load(cp, (128, 128), F16, ident16.ap(), "id16")
        id32 = load(cp, (128, 128), F32, ident32.ap(), "id32")
        ilo = load(cp, (128, NWIN * SL // 16), I16, idxlo.ap(), "ilo")
        ihi = load(cp, (128, NWIN * SH // 16), I16, idxhi.ap(), "ihi")
        dl_t = load(cp, (128, NWIN * C), F16, dstloc.ap(), "dlt")
        Wemb_t = load(cp, (cfg.f_in, H), F32, W_emb.ap(), "wemb")
        Wl_t = [load(cp, (H, H), F32, Ws.ap()[i], tag=f"Wl{i}")
                for i in range(cfg.L)]
        bemb = load(cp, (H, 1), F32, bemb_t.ap(), "bemb")
        bsl = load(cp, (H, cfg.L), F32, bs_t.ap(), "bsl")
        gl = load(cp, (H, cfg.L), F32, g_t.ap(), "gl")
        bel = load(cp, (H, cfg.L), F32, be_t.ap(), "bel")
        W1a_t = load(cp, (H, H), F32, W1a.ap(), "w1a")
        W1b_t = load(cp, (H, H), F32, W1b.ap(), "w1b")
        W1c_t = load(cp, (cfg.tdim, H), F32, W1c.ap(), "w1c")
        b1 = load(cp, (H, 1), F32, b1_t.ap(), "b1")
        W2_t = load(cp, (H, H2), F32, W2.ap(), "w2")
        b2 = load(cp, (H2, 1), F32, b2_t.ap(), "b2")
        W3_t = load(cp, (H2, 1), F32, W3.ap(), "w3")
        b3 = load(cp, (1, 1), F32, b3_t.ap(), "b3")

        def cast16(src, shape, tag):
            t = cp.tile(list(shape), F16, tag=tag)
            nc.vector.tensor_copy(out=t[:], in_=src[:])
            return t
        W1a16 = cast16(W1a_t, (H, H), "w1a16")
        W1b16 = cast16(W1b_t, (H, H), "w1b16")
        W1c16 = cast16(W1c_t, (cfg.tdim, H), "w1c16")
        W216 = cast16(W2_t, (H, H2), "w216")
        W316 = cast16(W3_t, (H2, 1), "w316")

        epst = cp.tile([128, 1], F32, tag="eps")
        nc.vector.memset(epst[:], float(cfg.eps))

        # ---- dinv ----------------------------------------------------------
        deg_t = load(cp, (128, NWIN), I32, deg.ap(), "deg")
        degf = cp.tile([128, NWIN], F32, tag="degf")
        nc.vector.tensor_copy(out=degf[:], in_=deg_t[:])
        sq = cp.tile([128, NWIN], F32, tag="sqdeg")
        nc.scalar.activation(out=sq[:], in_=degf[:], func=AF.Sqrt)
        dinv_nm = cp.tile([128, NWIN], F32, tag="dinvnm")
        nc.vector.reciprocal(out=dinv_nm[:], in_=sq[:])
        dinvd = nc.dram_tensor("dinvd", [SPAD], F32)
        nc.sync.dma_start(out=dinvd.ap().rearrange("(w p) -> p w", p=128),
                          in_=dinv_nm[:])

        with tc.tile_pool(name="big", bufs=1) as bigp:

            dinvrep = bigp.tile([128, SPAD], F32)
            nc.sync.dma_start(
                out=dinvrep[:],
                in_=dinvd.ap().unsqueeze(0).broadcast_to([128, SPAD]))

            stage = bigp.tile([128, SPAD], F16)
            nm = bigp.tile([128, SPAD], F16)
            vs = bigp.tile([128, SPAD], F32)
            hpre = bigp.tile([128, SPAD], F32)
            if SPAD > S:
                nc.vector.memset(stage[:, S:SPAD], 0.0)
                nc.vector.memset(vs[:, S:SPAD], 0.0)

            def zchunks():
                k = 0
                while k < SPAD:
                    yield k, min(512, SPAD - k)
                    k += 512

            # ---- embed -----------------------------------------------------
            with tc.tile_pool(name="emb", bufs=1) as embp:
                xload = embp.tile([128, NWIN * cfg.f_in], F32)
                nc.vector.memset(xload[:], 0.0)
                nfull = S // 128
                x3 = xload[:].rearrange("p (w f) -> p w f", f=cfg.f_in)
                nc.sync.dma_start(
                    out=x3[:, :nfull, :],
                    in_=xs.ap()[:nfull * 128, :]
                        .rearrange("(w p) f -> p w f", p=128))
                rem = S - nfull * 128
                if rem:
                    nc.sync.dma_start(
                        out=x3[:rem, nfull, :],
                        in_=xs.ap()[nfull * 128:, :]
                            .rearrange("(w p) f -> p w f", p=rem))
                xT = embp.tile([cfg.f_in, SPAD], F32)
                for w in range(NWIN):
                    pt = ptr.tile([cfg.f_in, 128], F32, tag="tr")
                    nc.tensor.transpose(
                        pt[:], xload[:, w * cfg.f_in:(w + 1) * cfg.f_in],
                        id32[:])
                    nc.vector.tensor_copy(out=xT[:, w * 128:(w + 1) * 128],
                                          in_=pt[:])
                for k, cw in zchunks():
                    pzt = pz.tile([128, 512], F32, tag="z")
                    nc.tensor.matmul(pzt[:, :cw], lhsT=Wemb_t[:],
                                     rhs=xT[:, k:k + cw],
                                     start=True, stop=True)
                    nc.scalar.activation(out=hpre[:, k:k + cw],
                                         in_=pzt[:, :cw],
                                         func=AF.Relu, bias=bemb[:])
                nc.vector.tensor_tensor(out=stage[:, :S], in0=hpre[:, :S],
                                        in1=dinvrep[:, :S], op=ALU.mult)

            def stage_to_T():
                for w in range(NWIN):
                    pt2 = ptr.tile([128, 128], F16, tag="tr")
                    nc.tensor.transpose(pt2[:],
                                        stage[:, w * 128:(w + 1) * 128],
                                        id16[:])
                    nc.vector.tensor_copy(out=nm[:, w * 128:(w + 1) * 128],
                                          in_=pt2[:])
                nc.sync.dma_start(
                    out=agin.ap().rearrange("(w p) f -> p w f", p=128),
                    in_=nm[:].rearrange("p (w f) -> p w f", f=H))
                nc.gpsimd.collective_compute(
                    "AllGather", ALU.bypass, replica_groups=RG,
                    ins=[agin.ap().opt()], outs=[T.ap().opt()])

            stage_to_T()

            # ---- layers ----------------------------------------------------
            T_low = T.ap()
            T_high = T.ap()[HALF:, :]
            iota3 = iota_t[:].rearrange("p (c e) -> p c e", e=128)

            msgp = tc.alloc_tile_pool(name="msg", bufs=3)
            ohp = tc.alloc_tile_pool(name="oh", bufs=3)
            for i in range(cfg.L):
                for w in range(NWIN):
                    ml = msgp.tile([128, CL, 128], F16, tag="mlo")
                    mh = msgp.tile([128, C - CL, 128], F16, tag="mhi")
                    MAXG = 1024
                    for g0 in range(0, SL, MAXG):
                        gn = min(MAXG, SL - g0)
                        nc.gpsimd.dma_gather(
                            ml[:, g0 // 128:(g0 + gn) // 128, :], T_low,
                            ilo[:, (w * SL + g0) // 16:(w * SL + g0 + gn) // 16],
                            gn, gn, H)
                    for g0 in range(0, SH, MAXG):
                        gn = min(MAXG, SH - g0)
                        nc.gpsimd.dma_gather(
                            mh[:, g0 // 128:(g0 + gn) // 128, :], T_high,
                            ihi[:, (w * SH + g0) // 16:(w * SH + g0 + gn) // 16],
                            gn, gn, H)
                    oh = ohp.tile([128, C, 128], F16, tag="oh")
                    nc.vector.tensor_tensor(
                        out=oh[:], in0=iota3,
                        in1=dl_t[:, w * C:(w + 1) * C].unsqueeze(2)
                            .broadcast_to([128, C, 128]),
                        op=ALU.is_equal)
                    pw = paggr.tile([128, 128], F32, tag="agg")
                    for c in range(C):
                        lhsT = ml[:, c, :] if c < CL else mh[:, c - CL, :]
                        nc.tensor.matmul(pw[:], lhsT=lhsT, rhs=oh[:, c, :],
                                         start=(c == 0), stop=(c == C - 1))
                    cw = min(128, S - w * 128)
                    sl = slice(w * 128, w * 128 + cw)
                    nc.vector.tensor_tensor(out=vs[:, sl], in0=pw[:, :cw],
                                            in1=stage[:, sl], op=ALU.add)
                    nc.vector.tensor_tensor(out=vs[:, sl], in0=vs[:, sl],
                                            in1=dinvrep[:, sl], op=ALU.mult)

                for k, cw in zchunks():
                    pzt = pz.tile([128, 512], F32, tag="z")
                    nc.tensor.matmul(pzt[:, :cw],
                                     lhsT=Wl_t[i][:],
                                     rhs=vs[:, k:k + cw],
                                     start=True, stop=True)
                    nc.scalar.activation(out=hpre[:, k:k + cw],
                                         in_=pzt[:, :cw],
                                         func=AF.Identity,
                                         bias=bsl[:, i:i + 1])

                # global BatchNorm stats
                sumt = smp.tile([128, 1], F32, tag="sum")
                sumsq = smp.tile([128, 1], F32, tag="sumsq")
                nc.vector.tensor_reduce(sumt[:], hpre[:, :S],
                                        axis=mybir.AxisListType.X, op=ALU.add)
                nc.vector.scalar_tensor_tensor(out=vs[:, :S], in0=hpre[:, :S],
                                               scalar=1.0, in1=hpre[:, :S],
                                               op0=ALU.mult, op1=ALU.mult,
                                               accum_out=sumsq[:])
                stats = smp.tile([128, 2], F32, tag="stats")
                nc.vector.tensor_copy(out=stats[:, 0:1], in_=sumt[:])
                nc.vector.tensor_copy(out=stats[:, 1:2], in_=sumsq[:])
                nc.sync.dma_start(out=stin.ap(), in_=stats[:])
                nc.gpsimd.collective_compute(
                    "AllReduce", ALU.add, replica_groups=RG,
                    ins=[stin.ap().opt()], outs=[stout.ap().opt()])
                stg = smp.tile([128, 2], F32, tag="stg")
                nc.sync.dma_start(out=stg[:], in_=stout.ap())

                mu = smp.tile([128, 1], F32, tag="mu")
                nc.vector.tensor_scalar(out=mu[:], in0=stg[:, 0:1],
                                        scalar1=1.0 / cfg.n, scalar2=None,
                                        op0=ALU.mult)
                musq = smp.tile([128, 1], F32, tag="musq")
                nc.vector.tensor_tensor(out=musq[:], in0=mu[:], in1=mu[:],
                                        op=ALU.mult)
                var = smp.tile([128, 1], F32, tag="var")
                nc.vector.scalar_tensor_tensor(out=var[:], in0=stg[:, 1:2],
                                               scalar=1.0 / cfg.n,
                                               in1=musq[:],
                                               op0=ALU.mult, op1=ALU.subtract)
                std = smp.tile([128, 1], F32, tag="std")
                nc.scalar.activation(out=std[:], in_=var[:], func=AF.Sqrt,
                                     bias=epst[:])
                rstd = smp.tile([128, 1], F32, tag="rstd")
                nc.vector.reciprocal(out=rstd[:], in_=std[:])
                scl = smp.tile([128, 1], F32, tag="scl")
                nc.vector.tensor_tensor(out=scl[:], in0=rstd[:],
                                        in1=gl[:, i:i + 1], op=ALU.mult)
                msc = smp.tile([128, 1], F32, tag="msc")
                nc.vector.tensor_tensor(out=msc[:], in0=mu[:], in1=scl[:],
                                        op=ALU.mult)
                shift = smp.tile([128, 1], F32, tag="shift")
                nc.vector.tensor_tensor(out=shift[:], in0=bel[:, i:i + 1],
                                        in1=msc[:], op=ALU.subtract)

                if i < cfg.L - 1:
                    nc.scalar.activation(out=vs[:, :S], in_=hpre[:, :S],
                                         func=AF.Relu, bias=shift[:],
                                         scale=scl[:])
                    nc.vector.tensor_tensor(out=stage[:, :S], in0=vs[:, :S],
                                            in1=dinvrep[:, :S], op=ALU.mult)
                else:
                    nc.scalar.activation(out=stage[:, :S], in_=hpre[:, :S],
                                         func=AF.Identity, bias=shift[:],
                                         scale=scl[:])
                stage_to_T()
            ohp.release()
            msgp.release()

        # ---- pair MLP (chunked) -------------------------------------------
        with tc.tile_pool(name="pair", bufs=2) as pp, \
             tc.tile_pool(name="pairc", bufs=1) as ppc:
            pidx_t = load(ppc, (128, 2 * BP // 16), I16, pidx.ap(), "pidx")
            T_low = T.ap()
            T_high = T.ap()[HALF:, :]
            boff = [0]
            for b in cfg.bsz:
                boff.append(boff[-1] + b)
            halves = [(0, 0), (0, 1), (1, 0), (1, 1)]
            PCH = cfg.pchunk
            for b in range(4):
                for k0 in range(boff[b], boff[b + 1], PCH):
                    cw = min(PCH, boff[b + 1] - k0)
                    nb = cw // 128
                    gs = pp.tile([128, PCH // 128, 128], F16, tag="gs")
                    gt = pp.tile([128, PCH // 128, 128], F16, tag="gt")
                    MAXG = 1024
                    for g0 in range(0, cw, MAXG):
                        gn = min(MAXG, cw - g0)
                        nc.gpsimd.dma_gather(
                            gs[:, g0 // 128:(g0 + gn) // 128, :],
                            T_high if halves[b][0] else T_low,
                            pidx_t[:, (k0 + g0) // 16:(k0 + g0 + gn) // 16],
                            gn, gn, H)
                        nc.gpsimd.dma_gather(
                            gt[:, g0 // 128:(g0 + gn) // 128, :],
                            T_high if halves[b][1] else T_low,
                            pidx_t[:, (BP + k0 + g0) // 16:(BP + k0 + g0 + gn) // 16],
                            gn, gn, H)
                    hsT = pp.tile([128, PCH], F16, tag="hsT")
                    htT = pp.tile([128, PCH], F16, tag="htT")
                    for t in range(nb):
                        for g, dstT in ((gs, hsT), (gt, htT)):
                            pt2 = ptr.tile([128, 128], F16, tag="tr")
                            nc.tensor.transpose(pt2[:], g[:, t, :], id16[:])
                            nc.vector.tensor_copy(
                                out=dstT[:, t * 128:(t + 1) * 128], in_=pt2[:])
                    tfc = pp.tile([cfg.tdim, PCH], F32, tag="tfc")
                    nc.sync.dma_start(out=tfc[:, :cw],
                                      in_=tfe.ap()[:, k0:k0 + cw])
                    tfc16 = pp.tile([cfg.tdim, PCH], F16, tag="tfc16")
                    nc.vector.tensor_copy(out=tfc16[:, :cw], in_=tfc[:, :cw])
                    y1T = pp.tile([128, PCH], F16, tag="y1T")
                    y2T = pp.tile([H2, PCH], F16, tag="y2T")
                    ysb = pp.tile([1, PCH], F32, tag="ysb")
                    for kk in range(0, cw, 512):
                        cc = min(512, cw - kk)
                        p1 = pz.tile([128, 512], F32, tag="z")
                        nc.tensor.matmul(p1[:, :cc], lhsT=W1a16[:],
                                         rhs=hsT[:, kk:kk + cc],
                                         start=True, stop=False)
                        nc.tensor.matmul(p1[:, :cc], lhsT=W1b16[:],
                                         rhs=htT[:, kk:kk + cc],
                                         start=False, stop=False)
                        nc.tensor.matmul(p1[:, :cc], lhsT=W1c16[:],
                                         rhs=tfc16[:, kk:kk + cc],
                                         start=False, stop=True)
                        nc.scalar.activation(out=y1T[:, kk:kk + cc],
                                             in_=p1[:, :cc],
                                             func=AF.Relu, bias=b1[:])
                        p2 = pz.tile([H2, 512], F32, tag="z")
                        nc.tensor.matmul(p2[:, :cc], lhsT=W216[:],
                                         rhs=y1T[:, kk:kk + cc],
                                         start=True, stop=True)
                        nc.scalar.activation(out=y2T[:, kk:kk + cc],
                                             in_=p2[:, :cc],
                                             func=AF.Relu, bias=b2[:])
                        p3 = pz.tile([1, 512], F32, tag="z")
                        nc.tensor.matmul(p3[:, :cc], lhsT=W316[:],
                                         rhs=y2T[:, kk:kk + cc],
                                         start=True, stop=True)
                        nc.scalar.activation(out=ysb[:, kk:kk + cc],
                                             in_=p3[:, :cc],
                                             func=AF.Identity, bias=b3[:])
                    nc.sync.dma_start(
                        out=yout.ap()[k0:k0 + cw].unsqueeze(0),
                        in_=ysb[:, :cw])

    nc.compile()
    return nc


# ---------------------------------------------------------------------------
# Entry point
# ---------------------------------------------------------------------------

_CACHE = {}


def kernel(**inputs):
    from concourse.bass_utils import run_bass_kernel_spmd

    cfg0 = Cfg()
    cfg, meta = host_prep(cfg0, inputs["edge_index"],
                          inputs["source_nodes"], inputs["target_nodes"])
    in_maps = host_inputs(cfg, meta, inputs)
    nc = _CACHE.get(cfg)
    if nc is None:
        nc = build_program(cfg, debug=False)
        _CACHE[cfg] = nc
    res = run_bass_kernel_spmd(nc, in_maps, list(range(cfg.ncores)))
    return assemble_output(cfg, meta, res.results)



# revision 4
# speedup vs baseline: 2.3077x; 2.3077x over previous
"""BikeFlowGNN Trainium2 kernel (self-contained).

kernel(**inputs) takes the FULL unsharded inputs (as produced by the
problem's setup_inputs) and returns the FULL [100000] output, running a
Bass/Tile SPMD program on 8 NeuronCores.

Sharding: nodes partitioned contiguously across the 8 cores (graph
parallel). Per GCN layer each core:
  - holds a replicated fp16 table T of u = dinv*h (node-major, per-rank
    row padding so every shard is window-aligned)
  - dma_gathers its edges' source rows (edges dst-sorted into 128-dst
    windows; int16 low/high index split at table row 32768)
  - aggregates messages with one-hot matmuls accumulating in PSUM
    (feature-major output), applies dinv, multiplies by W (fp32r),
    adds bias, BatchNorm with AllReduce'd stats, relu, rescales by dinv
  - PE-transposes its shard to node-major and AllGathers into T
The pair MLP shards the 100k prediction pairs (4 buckets by src/tgt
index half), processed in SBUF-sized chunks.
"""

import dataclasses
import numpy as np

import concourse.bacc as bacc
import concourse.tile as tile
import concourse.mybir as mybir

F32 = mybir.dt.float32
F32R = mybir.dt.float32r
F16 = mybir.dt.float16
I16 = mybir.dt.int16
I32 = mybir.dt.int32
AF = mybir.ActivationFunctionType
ALU = mybir.AluOpType


@dataclasses.dataclass(frozen=True)
class Cfg:
    n: int = 50000
    e: int = 1600000
    p: int = 100000
    f_in: int = 32
    h: int = 128
    tdim: int = 2
    L: int = 3
    eps: float = 1e-5
    ncores: int = 8
    half: int = 32768
    win: int = 128
    h2: int = 64
    pchunk: int = 2048
    # static padded sizes (from host prep)
    s_low: int = 0
    s_high: int = 0
    bsz: tuple = ()

    @property
    def s_nodes(self):
        return self.n // self.ncores

    @property
    def nwin(self):
        return (self.s_nodes + self.win - 1) // self.win

    @property
    def spad(self):
        return self.nwin * self.win

    @property
    def tpad(self):
        return self.spad * self.ncores

    @property
    def C(self):
        return (self.s_low + self.s_high) // 128

    @property
    def CL(self):
        return self.s_low // 128

    @property
    def bp(self):
        return sum(self.bsz)


# ---------------------------------------------------------------------------
# Host preprocessing (index manipulation only)
# ---------------------------------------------------------------------------

def _wrap16(a):
    """[..., k] -> [..., 16, k/16] with element i at [i%16, i//16]."""
    assert a.shape[-1] % 16 == 0
    return np.ascontiguousarray(
        a.reshape(a.shape[:-1] + (a.shape[-1] // 16, 16)).swapaxes(-1, -2))


def host_prep(cfg0: Cfg, edge_index, source_nodes, target_nodes):
    S = cfg0.s_nodes
    NWIN, WIN, NC = cfg0.nwin, cfg0.win, cfg0.ncores
    SPAD, HALF = cfg0.spad, cfg0.half

    src = np.asarray(edge_index[0], np.int64)
    dst = np.asarray(edge_index[1], np.int64)

    # Degree-balanced node relabeling: deal nodes (sorted by in-degree)
    # round-robin across the (core, window) slots so every window sees a
    # near-equal edge count -> minimal static gather padding.
    deg_in = np.bincount(dst, minlength=cfg0.n)
    order_nodes = np.argsort(-deg_in, kind="stable")
    caps = np.full(NC * NWIN, WIN, np.int64)
    caps[NWIN - 1::NWIN] = S - (NWIN - 1) * WIN
    perm = np.empty(cfg0.n, np.int64)
    taken = 0
    for rnd in range(int(caps.max())):
        live = np.nonzero(caps > rnd)[0]
        k = min(len(live), cfg0.n - taken)
        nodes = order_nodes[taken:taken + k]
        taken += k
        lv = live[:k]
        perm[nodes] = (lv // NWIN) * S + (lv % NWIN) * WIN + rnd
        if taken >= cfg0.n:
            break
    assert taken == cfg0.n
    src = perm[src]
    dst = perm[dst]

    trow = (src // S) * SPAD + (src % S)
    lo = trow < HALF
    core = dst // S
    win = (dst % S) // WIN
    dloc = (dst % S) - win * WIN

    grp = (core * NWIN + win) * 2 + (1 - lo.astype(np.int64))
    ngrp = NC * NWIN * 2
    cnt = np.bincount(grp, minlength=ngrp).reshape(NC, NWIN, 2)
    s_low = max(128, int(np.ceil(cnt[:, :, 0].max() / 128) * 128))
    s_high = max(128, int(np.ceil(cnt[:, :, 1].max() / 128) * 128))
    C = (s_low + s_high) // 128

    order = np.argsort(grp, kind="stable")
    grp_s = grp[order]
    starts = np.zeros(ngrp + 1, np.int64)
    np.cumsum(np.bincount(grp_s, minlength=ngrp), out=starts[1:])
    within = np.arange(len(grp_s)) - starts[grp_s]

    glo = grp_s % 2 == 0
    gc = grp_s // (2 * NWIN)
    gw = (grp_s // 2) % NWIN

    idx_low = np.zeros((NC, NWIN, s_low), np.int16)
    idx_high = np.zeros((NC, NWIN, s_high), np.int16)
    dstloc = np.full((NC, NWIN, C * 128), -1.0, np.float16)

    tr_s, dl_s = trow[order], dloc[order]
    m = glo
    idx_low[gc[m], gw[m], within[m]] = tr_s[m].astype(np.int16)
    dstloc[gc[m], gw[m], within[m]] = dl_s[m].astype(np.float16)
    m = ~glo
    idx_high[gc[m], gw[m], within[m]] = (tr_s[m] - HALF).astype(np.int16)
    dstloc[gc[m], gw[m], s_low + within[m]] = dl_s[m].astype(np.float16)

    idxlo_img = np.tile(_wrap16(idx_low).transpose(0, 2, 1, 3)
                        .reshape(NC, 16, NWIN * s_low // 16), (1, 8, 1))
    idxhi_img = np.tile(_wrap16(idx_high).transpose(0, 2, 1, 3)
                        .reshape(NC, 16, NWIN * s_high // 16), (1, 8, 1))
    dst_img = dstloc.reshape(NC, NWIN, C, 128).transpose(0, 3, 1, 2) \
                    .reshape(NC, 128, NWIN * C)

    # ---- pairs: 4 buckets by (src-half, tgt-half)
    PC = cfg0.p // NC
    sn = perm[np.asarray(source_nodes, np.int64)].reshape(NC, PC)
    tn = perm[np.asarray(target_nodes, np.int64)].reshape(NC, PC)
    srow = (sn // S) * SPAD + (sn % S)
    trow_p = (tn // S) * SPAD + (tn % S)
    bucket = (srow >= HALF).astype(np.int64) * 2 + (trow_p >= HALF).astype(np.int64)
    bsz = []
    for b in range(4):
        bsz.append(max(128, int(np.ceil((bucket == b).sum(1).max() / 128) * 128)))
    BP = sum(bsz)
    boff = np.concatenate([[0], np.cumsum(bsz)])

    pidx = np.zeros((NC, 2, BP), np.int16)
    ppos = np.zeros((NC, PC), np.int64)
    for c in range(NC):
        o = np.argsort(bucket[c], kind="stable")
        bc = bucket[c][o]
        within = np.arange(PC) - np.searchsorted(bc, bc)
        pos = boff[bc] + within
        sr, tr = srow[c][o], trow_p[c][o]
        pidx[c, 0, pos] = np.where(sr < HALF, sr, sr - HALF).astype(np.int16)
        pidx[c, 1, pos] = np.where(tr < HALF, tr, tr - HALF).astype(np.int16)
        ppos[c, o] = pos
    pidx_img = np.stack([
        np.tile(np.concatenate([_wrap16(pidx[c, 0]), _wrap16(pidx[c, 1])],
                               axis=1), (8, 1))
        for c in range(NC)])

    cfg = dataclasses.replace(cfg0, s_low=s_low, s_high=s_high, bsz=tuple(bsz))
    meta = dict(idxlo=idxlo_img, idxhi=idxhi_img, dstloc=dst_img,
                pidx=pidx_img, ppos=ppos, boff=boff, perm=perm)
    return cfg, meta


def host_inputs(cfg: Cfg, meta, inputs):
    NC, S = cfg.ncores, cfg.s_nodes
    perm = meta["perm"]
    dst = perm[np.asarray(inputs["edge_index"][1], np.int64)]
    deg = (np.bincount(dst, minlength=cfg.n) + 1).astype(np.int32)
    degp = np.ones((NC, cfg.spad), np.int32)
    degp[:, :S] = deg.reshape(NC, S)
    deg_img = degp.reshape(NC, cfg.nwin, 128).transpose(0, 2, 1)  # [NC,128,NWIN]

    iota = np.tile(np.arange(128, dtype=np.float16)[None, :], (128, cfg.C))
    W1 = np.asarray(inputs["W1"], np.float32)
    PC = cfg.p // NC
    tf = np.asarray(inputs["time_feats"], np.float32).reshape(NC, PC, cfg.tdim)

    common = dict(
        iota=iota,
        ident16=np.eye(128, dtype=np.float16),
        ident32=np.eye(128, dtype=np.float32),
        W_emb=np.asarray(inputs["W_emb"], np.float32),
        Ws=np.asarray(inputs["Ws"], np.float32),
        bemb_t=np.asarray(inputs["b_emb"], np.float32).reshape(cfg.h, 1),
        bs_t=np.ascontiguousarray(np.asarray(inputs["bs"], np.float32).T),
        g_t=np.ascontiguousarray(np.asarray(inputs["gammas"], np.float32).T),
        be_t=np.ascontiguousarray(np.asarray(inputs["betas"], np.float32).T),
        W1a=np.ascontiguousarray(W1[:cfg.h]),
        W1b=np.ascontiguousarray(W1[cfg.h:2 * cfg.h]),
        W1c=np.ascontiguousarray(W1[2 * cfg.h:]),
        b1_t=np.asarray(inputs["b1"], np.float32).reshape(cfg.h, 1),
        W2=np.asarray(inputs["W2"], np.float32),
        b2_t=np.asarray(inputs["b2"], np.float32).reshape(cfg.h2, 1),
        W3=np.asarray(inputs["W3"], np.float32),
        b3_t=np.asarray(inputs["b3"], np.float32).reshape(1, 1),
    )

    x_old = np.asarray(inputs["x"], np.float32)
    x = np.empty_like(x_old)
    x[perm] = x_old
    in_maps = []
    for c in range(NC):
        tfe = np.zeros((cfg.tdim, cfg.bp), np.float32)
        tfe[:, meta["ppos"][c]] = tf[c].T
        m = dict(common)
        m.update(
            xs=np.ascontiguousarray(x[c * S:(c + 1) * S]),
            deg=np.ascontiguousarray(deg_img[c]),
            idxlo=meta["idxlo"][c], idxhi=meta["idxhi"][c],
            dstloc=meta["dstloc"][c], pidx=meta["pidx"][c],
            tfe=tfe,
        )
        in_maps.append(m)
    return in_maps


def assemble_output(cfg: Cfg, meta, results):
    NC, PC = cfg.ncores, cfg.p // cfg.ncores
    y = np.zeros(cfg.p, np.float32)
    for c in range(NC):
        y[c * PC:(c + 1) * PC] = results[c]["yout"][meta["ppos"][c]]
    return y


# ---------------------------------------------------------------------------
# Bass program
# ---------------------------------------------------------------------------

def build_program(cfg: Cfg, debug=False):
    NC = cfg.ncores
    H, H2, S, SPAD, NWIN, C, CL = (cfg.h, cfg.h2, cfg.s_nodes, cfg.spad,
                                   cfg.nwin, cfg.C, cfg.CL)
    SL, SH, HALF = cfg.s_low, cfg.s_high, cfg.half
    BP = cfg.bp
    RG = [list(range(NC))]

    nc = bacc.Bacc("TRN2", target_bir_lowering=False, debug=debug,
                   num_devices=NC, num_swdge_queues=4)
    qctr = [0]

    def next_q():
        q = qctr[0] % 4
        qctr[0] += 1
        return q

    def param(name, shape, dt_):
        return nc.dram_tensor(name, list(shape), dt_, kind="ExternalInput")

    xs = param("xs", (S, cfg.f_in), F32)
    deg = param("deg", (128, NWIN), I32)
    idxlo = param("idxlo", (128, NWIN * SL // 16), I16)
    idxhi = param("idxhi", (128, NWIN * SH // 16), I16)
    dstloc = param("dstloc", (128, NWIN * C), F16)
    iota = param("iota", (128, C * 128), F16)
    ident16 = param("ident16", (128, 128), F16)
    ident32 = param("ident32", (128, 128), F32)
    W_emb = param("W_emb", (cfg.f_in, H), F32)
    Ws = param("Ws", (cfg.L, H, H), F32)
    bemb_t = param("bemb_t", (H, 1), F32)
    bs_t = param("bs_t", (H, cfg.L), F32)
    g_t = param("g_t", (H, cfg.L), F32)
    be_t = param("be_t", (H, cfg.L), F32)
    W1a = param("W1a", (H, H), F32)
    W1b = param("W1b", (H, H), F32)
    W1c = param("W1c", (cfg.tdim, H), F32)
    b1_t = param("b1_t", (H, 1), F32)
    W2 = param("W2", (H, H2), F32)
    b2_t = param("b2_t", (H2, 1), F32)
    W3 = param("W3", (H2, 1), F32)
    b3_t = param("b3_t", (1, 1), F32)
    pidx = param("pidx", (128, 2 * BP // 16), I16)
    tfe = param("tfe", (cfg.tdim, BP), F32)

    yout = nc.dram_tensor("yout", [BP], F32, kind="ExternalOutput")

    T = nc.dram_tensor("Tbl", [cfg.tpad, H], F16, addr_space="Shared")
    agin = nc.dram_tensor("agin", [SPAD, H], F16)
    stin = nc.dram_tensor("stin", [H, 2], F32)
    stout = nc.dram_tensor("stout", [H, 2], F32, addr_space="Shared")

    with tile.TileContext(nc) as tc, \
         tc.tile_pool(name="const", bufs=1) as cp, \
         tc.tile_pool(name="sm", bufs=2) as smp, \
         tc.tile_pool(name="paggr", bufs=3, space="PSUM") as paggr, \
         tc.tile_pool(name="ptr", bufs=2, space="PSUM") as ptr, \
         tc.tile_pool(name="pz", bufs=2, space="PSUM") as pz:

        def load(p, shape, dt_, src, tag):
            t = p.tile(list(shape), dt_, tag=tag)
            nc.sync.dma_start(out=t[:], in_=src)
            return t

        iota_t = load(cp, (128, C * 128), F16, iota.ap(), "iota")
        id16 = load(cp, (128, 128), F16, ident16.ap(), "id16")
        id32 = load(cp, (128, 128), F32, ident32.ap(), "id32")
        ilo = load(cp, (128, NWIN * SL // 16), I16, idxlo.ap(), "ilo")
        ihi = load(cp, (128, NWIN * SH // 16), I16, idxhi.ap(), "ihi")
        dl_t = load(cp, (128, NWIN * C), F16, dstloc.ap(), "dlt")
        Wemb_t = load(cp, (cfg.f_in, H), F32, W_emb.ap(), "wemb")
        Wl_t = [load(cp, (H, H), F32, Ws.ap()[i], tag=f"Wl{i}")
                for i in range(cfg.L)]
        bemb = load(cp, (H, 1), F32, bemb_t.ap(), "bemb")
        bsl = load(cp, (H, cfg.L), F32, bs_t.ap(), "bsl")
        gl = load(cp, (H, cfg.L), F32, g_t.ap(), "gl")
        bel = load(cp, (H, cfg.L), F32, be_t.ap(), "bel")
        W1a_t = load(cp, (H, H), F32, W1a.ap(), "w1a")
        W1b_t = load(cp, (H, H), F32, W1b.ap(), "w1b")
        W1c_t = load(cp, (cfg.tdim, H), F32, W1c.ap(), "w1c")
        b1 = load(cp, (H, 1), F32, b1_t.ap(), "b1")
        W2_t = load(cp, (H, H2), F32, W2.ap(), "w2")
        b2 = load(cp, (H2, 1), F32, b2_t.ap(), "b2")
        W3_t = load(cp, (H2, 1), F32, W3.ap(), "w3")
        b3 = load(cp, (1, 1), F32, b3_t.ap(), "b3")

        def cast16(src, shape, tag):
            t = cp.tile(list(shape), F16, tag=tag)
            nc.vector.tensor_copy(out=t[:], in_=src[:])
            return t
        W1a16 = cast16(W1a_t, (H, H), "w1a16")
        W1b16 = cast16(W1b_t, (H, H), "w1b16")
        W1c16 = cast16(W1c_t, (cfg.tdim, H), "w1c16")
        W216 = cast16(W2_t, (H, H2), "w216")
        W316 = cast16(W3_t, (H2, 1), "w316")

        epst = cp.tile([128, 1], F32, tag="eps")
        nc.vector.memset(epst[:], float(cfg.eps))

        # ---- dinv ----------------------------------------------------------
        deg_t = load(cp, (128, NWIN), I32, deg.ap(), "deg")
        degf = cp.tile([128, NWIN], F32, tag="degf")
        nc.vector.tensor_copy(out=degf[:], in_=deg_t[:])
        sq = cp.tile([128, NWIN], F32, tag="sqdeg")
        nc.scalar.activation(out=sq[:], in_=degf[:], func=AF.Sqrt)
        dinv_nm = cp.tile([128, NWIN], F32, tag="dinvnm")
        nc.vector.reciprocal(out=dinv_nm[:], in_=sq[:])
        dinvd = nc.dram_tensor("dinvd", [SPAD], F32)
        nc.sync.dma_start(out=dinvd.ap().rearrange("(w p) -> p w", p=128),
                          in_=dinv_nm[:])

        with tc.tile_pool(name="big", bufs=1) as bigp:

            dinvrep = bigp.tile([128, SPAD], F32)
            nc.sync.dma_start(
                out=dinvrep[:],
                in_=dinvd.ap().unsqueeze(0).broadcast_to([128, SPAD]))

            stage = bigp.tile([128, SPAD], F16)
            nm = bigp.tile([128, SPAD], F16)
            vs = bigp.tile([128, SPAD], F32)
            hpre = bigp.tile([128, SPAD], F32)
            if SPAD > S:
                nc.vector.memset(stage[:, S:SPAD], 0.0)
                nc.vector.memset(vs[:, S:SPAD], 0.0)

            def zchunks():
                k = 0
                while k < SPAD:
                    yield k, min(512, SPAD - k)
                    k += 512

            # ---- embed -----------------------------------------------------
            with tc.tile_pool(name="emb", bufs=1) as embp:
                xload = embp.tile([128, NWIN * cfg.f_in], F32)
                nc.vector.memset(xload[:], 0.0)
                nfull = S // 128
                x3 = xload[:].rearrange("p (w f) -> p w f", f=cfg.f_in)
                nc.sync.dma_start(
                    out=x3[:, :nfull, :],
                    in_=xs.ap()[:nfull * 128, :]
                        .rearrange("(w p) f -> p w f", p=128))
                rem = S - nfull * 128
                if rem:
                    nc.sync.dma_start(
                        out=x3[:rem, nfull, :],
                        in_=xs.ap()[nfull * 128:, :]
                            .rearrange("(w p) f -> p w f", p=rem))
                xT = embp.tile([cfg.f_in, SPAD], F32)
                for w in range(NWIN):
                    pt = ptr.tile([cfg.f_in, 128], F32, tag="tr")
                    nc.tensor.transpose(
                        pt[:], xload[:, w * cfg.f_in:(w + 1) * cfg.f_in],
                        id32[:])
                    nc.vector.tensor_copy(out=xT[:, w * 128:(w + 1) * 128],
                                          in_=pt[:])
                for k, cw in zchunks():
                    pzt = pz.tile([128, 512], F32, tag="z")
                    nc.tensor.matmul(pzt[:, :cw], lhsT=Wemb_t[:],
                                     rhs=xT[:, k:k + cw],
                                     start=True, stop=True)
                    nc.scalar.activation(out=hpre[:, k:k + cw],
                                         in_=pzt[:, :cw],
                                         func=AF.Relu, bias=bemb[:])
                nc.vector.tensor_tensor(out=stage[:, :S], in0=hpre[:, :S],
                                        in1=dinvrep[:, :S], op=ALU.mult)

            def stage_to_T():
                for w in range(NWIN):
                    pt2 = ptr.tile([128, 128], F16, tag="tr")
                    nc.tensor.transpose(pt2[:],
                                        stage[:, w * 128:(w + 1) * 128],
                                        id16[:])
                    nc.vector.tensor_copy(out=nm[:, w * 128:(w + 1) * 128],
                                          in_=pt2[:])
                nc.sync.dma_start(
                    out=agin.ap().rearrange("(w p) f -> p w f", p=128),
                    in_=nm[:].rearrange("p (w f) -> p w f", f=H))
                nc.gpsimd.collective_compute(
                    "AllGather", ALU.bypass, replica_groups=RG,
                    ins=[agin.ap().opt()], outs=[T.ap().opt()])

            stage_to_T()

            # ---- layers ----------------------------------------------------
            T_low = T.ap()
            T_high = T.ap()[HALF:, :]
            iota3 = iota_t[:].rearrange("p (c e) -> p c e", e=128)

            msgp = tc.alloc_tile_pool(name="msg", bufs=3)
            ohp = tc.alloc_tile_pool(name="oh", bufs=3)
            for i in range(cfg.L):
                for w in range(NWIN):
                    ml = msgp.tile([128, CL, 128], F16, tag="mlo")
                    mh = msgp.tile([128, C - CL, 128], F16, tag="mhi")
                    MAXG = 1024
                    for g0 in range(0, SL, MAXG):
                        gn = min(MAXG, SL - g0)
                        nc.gpsimd.dma_gather(
                            ml[:, g0 // 128:(g0 + gn) // 128, :], T_low,
                            ilo[:, (w * SL + g0) // 16:(w * SL + g0 + gn) // 16],
                            gn, gn, H, queue_num=next_q())
                    for g0 in range(0, SH, MAXG):
                        gn = min(MAXG, SH - g0)
                        nc.gpsimd.dma_gather(
                            mh[:, g0 // 128:(g0 + gn) // 128, :], T_high,
                            ihi[:, (w * SH + g0) // 16:(w * SH + g0 + gn) // 16],
                            gn, gn, H, queue_num=next_q())
                    oh = ohp.tile([128, C, 128], F16, tag="oh")
                    nc.vector.tensor_tensor(
                        out=oh[:], in0=iota3,
                        in1=dl_t[:, w * C:(w + 1) * C].unsqueeze(2)
                            .broadcast_to([128, C, 128]),
                        op=ALU.is_equal)
                    pw = paggr.tile([128, 128], F32, tag="agg")
                    for c in range(C):
                        lhsT = ml[:, c, :] if c < CL else mh[:, c - CL, :]
                        nc.tensor.matmul(pw[:], lhsT=lhsT, rhs=oh[:, c, :],
                                         start=(c == 0), stop=(c == C - 1))
                    cw = min(128, S - w * 128)
                    sl = slice(w * 128, w * 128 + cw)
                    nc.vector.tensor_tensor(out=vs[:, sl], in0=pw[:, :cw],
                                            in1=stage[:, sl], op=ALU.add)
                    nc.vector.tensor_tensor(out=vs[:, sl], in0=vs[:, sl],
                                            in1=dinvrep[:, sl], op=ALU.mult)

                for k, cw in zchunks():
                    pzt = pz.tile([128, 512], F32, tag="z")
                    nc.tensor.matmul(pzt[:, :cw],
                                     lhsT=Wl_t[i][:],
                                     rhs=vs[:, k:k + cw],
                                     start=True, stop=True)
                    nc.scalar.activation(out=hpre[:, k:k + cw],
                                         in_=pzt[:, :cw],
                                         func=AF.Identity,
                                         bias=bsl[:, i:i + 1])

                # global BatchNorm stats
                sumt = smp.tile([128, 1], F32, tag="sum")
                sumsq = smp.tile([128, 1], F32, tag="sumsq")
                nc.vector.tensor_reduce(sumt[:], hpre[:, :S],
                                        axis=mybir.AxisListType.X, op=ALU.add)
                nc.vector.scalar_tensor_tensor(out=vs[:, :S], in0=hpre[:, :S],
                                               scalar=1.0, in1=hpre[:, :S],
                                               op0=ALU.mult, op1=ALU.mult,
                                               accum_out=sumsq[:])
                stats = smp.tile([128, 2], F32, tag="stats")
                nc.vector.tensor_copy(out=stats[:, 0:1], in_=sumt[:])
                nc.vector.tensor_copy(out=stats[:, 1:2], in_=sumsq[:])
                nc.sync.dma_start(out=stin.ap(), in_=stats[:])
                nc.gpsimd.collective_compute(
                    "AllReduce", ALU.add, replica_groups=RG,
                    ins=[stin.ap().opt()], outs=[stout.ap().opt()])
                stg = smp.tile([128, 2], F32, tag="stg")
                nc.sync.dma_start(out=stg[:], in_=stout.ap())

                mu = smp.tile([128, 1], F32, tag="mu")
                nc.vector.tensor_scalar(out=mu[:], in0=stg[:, 0:1],
                                        scalar1=1.0 / cfg.n, scalar2=None,
                                        op0=ALU.mult)
                musq = smp.tile([128, 1], F32, tag="musq")
                nc.vector.tensor_tensor(out=musq[:], in0=mu[:], in1=mu[:],
                                        op=ALU.mult)
                var = smp.tile([128, 1], F32, tag="var")
                nc.vector.scalar_tensor_tensor(out=var[:], in0=stg[:, 1:2],
                                               scalar=1.0 / cfg.n,
                                               in1=musq[:],
                                               op0=ALU.mult, op1=ALU.subtract)
                std = smp.tile([128, 1], F32, tag="std")
                nc.scalar.activation(out=std[:], in_=var[:], func=AF.Sqrt,
                                     bias=epst[:])
                rstd = smp.tile([128, 1], F32, tag="rstd")
                nc.vector.reciprocal(out=rstd[:], in_=std[:])
                scl = smp.tile([128, 1], F32, tag="scl")
                nc.vector.tensor_tensor(out=scl[:], in0=rstd[:],
                                        in1=gl[:, i:i + 1], op=ALU.mult)
                msc = smp.tile([128, 1], F32, tag="msc")
                nc.vector.tensor_tensor(out=msc[:], in0=mu[:], in1=scl[:],
                                        op=ALU.mult)
                shift = smp.tile([128, 1], F32, tag="shift")
                nc.vector.tensor_tensor(out=shift[:], in0=bel[:, i:i + 1],
                                        in1=msc[:], op=ALU.subtract)

                if i < cfg.L - 1:
                    nc.scalar.activation(out=vs[:, :S], in_=hpre[:, :S],
                                         func=AF.Relu, bias=shift[:],
                                         scale=scl[:])
                    nc.vector.tensor_tensor(out=stage[:, :S], in0=vs[:, :S],
                                            in1=dinvrep[:, :S], op=ALU.mult)
                else:
                    nc.scalar.activation(out=stage[:, :S], in_=hpre[:, :S],
                                         func=AF.Identity, bias=shift[:],
                                         scale=scl[:])
                stage_to_T()
            ohp.release()
            msgp.release()

        # ---- pair MLP (chunked) -------------------------------------------
        with tc.tile_pool(name="pair", bufs=2) as pp, \
             tc.tile_pool(name="pairc", bufs=1) as ppc:
            pidx_t = load(ppc, (128, 2 * BP // 16), I16, pidx.ap(), "pidx")
            T_low = T.ap()
            T_high = T.ap()[HALF:, :]
            boff = [0]
            for b in cfg.bsz:
                boff.append(boff[-1] + b)
            halves = [(0, 0), (0, 1), (1, 0), (1, 1)]
            PCH = cfg.pchunk
            for b in range(4):
                for k0 in range(boff[b], boff[b + 1], PCH):
                    cw = min(PCH, boff[b + 1] - k0)
                    nb = cw // 128
                    gs = pp.tile([128, PCH // 128, 128], F16, tag="gs")
                    gt = pp.tile([128, PCH // 128, 128], F16, tag="gt")
                    MAXG = 1024
                    for g0 in range(0, cw, MAXG):
                        gn = min(MAXG, cw - g0)
                        nc.gpsimd.dma_gather(
                            gs[:, g0 // 128:(g0 + gn) // 128, :],
                            T_high if halves[b][0] else T_low,
                            pidx_t[:, (k0 + g0) // 16:(k0 + g0 + gn) // 16],
                            gn, gn, H, queue_num=next_q())
                        nc.gpsimd.dma_gather(
                            gt[:, g0 // 128:(g0 + gn) // 128, :],
                            T_high if halves[b][1] else T_low,
                            pidx_t[:, (BP + k0 + g0) // 16:(BP + k0 + g0 + gn) // 16],
                            gn, gn, H, queue_num=next_q())
                    hsT = pp.tile([128, PCH], F16, tag="hsT")
                    htT = pp.tile([128, PCH], F16, tag="htT")
                    for t in range(nb):
                        for g, dstT in ((gs, hsT), (gt, htT)):
                            pt2 = ptr.tile([128, 128], F16, tag="tr")
                            nc.tensor.transpose(pt2[:], g[:, t, :], id16[:])
                            nc.vector.tensor_copy(
                                out=dstT[:, t * 128:(t + 1) * 128], in_=pt2[:])
                    tfc = pp.tile([cfg.tdim, PCH], F32, tag="tfc")
                    nc.sync.dma_start(out=tfc[:, :cw],
                                      in_=tfe.ap()[:, k0:k0 + cw])
                    tfc16 = pp.tile([cfg.tdim, PCH], F16, tag="tfc16")
                    nc.vector.tensor_copy(out=tfc16[:, :cw], in_=tfc[:, :cw])
                    y1T = pp.tile([128, PCH], F16, tag="y1T")
                    y2T = pp.tile([H2, PCH], F16, tag="y2T")
                    ysb = pp.tile([1, PCH], F32, tag="ysb")
                    for kk in range(0, cw, 512):
                        cc = min(512, cw - kk)
                        p1 = pz.tile([128, 512], F32, tag="z")
                        nc.tensor.matmul(p1[:, :cc], lhsT=W1a16[:],
                                         rhs=hsT[:, kk:kk + cc],
                                         start=True, stop=False)
                        nc.tensor.matmul(p1[:, :cc], lhsT=W1b16[:],
                                         rhs=htT[:, kk:kk + cc],
                                         start=False, stop=False)
                        nc.tensor.matmul(p1[:, :cc], lhsT=W1c16[:],
                                         rhs=tfc16[:, kk:kk + cc],
                                         start=False, stop=True)
                        nc.scalar.activation(out=y1T[:, kk:kk + cc],
                                             in_=p1[:, :cc],
                                             func=AF.Relu, bias=b1[:])
                        p2 = pz.tile([H2, 512], F32, tag="z")
                        nc.tensor.matmul(p2[:, :cc], lhsT=W216[:],
                                         rhs=y1T[:, kk:kk + cc],
                                         start=True, stop=True)
                        nc.scalar.activation(out=y2T[:, kk:kk + cc],
                                             in_=p2[:, :cc],
                                             func=AF.Relu, bias=b2[:])
                        p3 = pz.tile([1, 512], F32, tag="z")
                        nc.tensor.matmul(p3[:, :cc], lhsT=W316[:],
                                         rhs=y2T[:, kk:kk + cc],
                                         start=True, stop=True)
                        nc.scalar.activation(out=ysb[:, kk:kk + cc],
                                             in_=p3[:, :cc],
                                             func=AF.Identity, bias=b3[:])
                    nc.sync.dma_start(
                        out=yout.ap()[k0:k0 + cw].unsqueeze(0),
                        in_=ysb[:, :cw])

    nc.compile()
    return nc


# ---------------------------------------------------------------------------
# Entry point
# ---------------------------------------------------------------------------

_CACHE = {}


def kernel(**inputs):
    from concourse.bass_utils import run_bass_kernel_spmd

    cfg0 = Cfg()
    cfg, meta = host_prep(cfg0, inputs["edge_index"],
                          inputs["source_nodes"], inputs["target_nodes"])
    in_maps = host_inputs(cfg, meta, inputs)
    nc = _CACHE.get(cfg)
    if nc is None:
        nc = build_program(cfg, debug=False)
        _CACHE[cfg] = nc
    res = run_bass_kernel_spmd(nc, in_maps, list(range(cfg.ncores)))
    return assemble_output(cfg, meta, res.results)



# revision 11
# speedup vs baseline: 2.3855x; 1.0337x over previous
"""BikeFlowGNN Trainium2 kernel (self-contained).

kernel(**inputs) takes the FULL unsharded inputs (as produced by the
problem's setup_inputs) and returns the FULL [100000] output, running a
Bass/Tile SPMD program on 8 NeuronCores.

Sharding: nodes partitioned contiguously across the 8 cores (graph
parallel). Per GCN layer each core:
  - holds a replicated fp16 table T of u = dinv*h (node-major, per-rank
    row padding so every shard is window-aligned)
  - dma_gathers its edges' source rows (edges dst-sorted into 128-dst
    windows; int16 low/high index split at table row 32768)
  - aggregates messages with one-hot matmuls accumulating in PSUM
    (feature-major output), applies dinv, multiplies by W (fp32r),
    adds bias, BatchNorm with AllReduce'd stats, relu, rescales by dinv
  - PE-transposes its shard to node-major and AllGathers into T
The pair MLP shards the 100k prediction pairs (4 buckets by src/tgt
index half), processed in SBUF-sized chunks.
"""

import dataclasses
import numpy as np

import concourse.bacc as bacc
import concourse.tile as tile
import concourse.mybir as mybir

F32 = mybir.dt.float32
F32R = mybir.dt.float32r
F16 = mybir.dt.float16
I16 = mybir.dt.int16
I32 = mybir.dt.int32
AF = mybir.ActivationFunctionType
ALU = mybir.AluOpType


@dataclasses.dataclass(frozen=True)
class Cfg:
    n: int = 50000
    e: int = 1600000
    p: int = 100000
    f_in: int = 32
    h: int = 128
    tdim: int = 2
    L: int = 3
    eps: float = 1e-5
    ncores: int = 8
    half: int = 32768
    win: int = 128
    h2: int = 64
    pchunk: int = 2048
    # static padded sizes (from host prep)
    s_low: int = 0
    s_high: int = 0
    bsz: tuple = ()

    @property
    def s_nodes(self):
        return self.n // self.ncores

    @property
    def nwin(self):
        return (self.s_nodes + self.win - 1) // self.win

    @property
    def spad(self):
        return self.nwin * self.win

    @property
    def tpad(self):
        return self.spad * self.ncores

    @property
    def C(self):
        return (self.s_low + self.s_high) // 128

    @property
    def CL(self):
        return self.s_low // 128

    @property
    def bp(self):
        return sum(self.bsz)


# ---------------------------------------------------------------------------
# Host preprocessing (index manipulation only)
# ---------------------------------------------------------------------------

def _wrap16(a):
    """[..., k] -> [..., 16, k/16] with element i at [i%16, i//16]."""
    assert a.shape[-1] % 16 == 0
    return np.ascontiguousarray(
        a.reshape(a.shape[:-1] + (a.shape[-1] // 16, 16)).swapaxes(-1, -2))


def host_prep(cfg0: Cfg, edge_index, source_nodes, target_nodes):
    S = cfg0.s_nodes
    NWIN, WIN, NC = cfg0.nwin, cfg0.win, cfg0.ncores
    SPAD, HALF = cfg0.spad, cfg0.half

    src = np.asarray(edge_index[0], np.int64)
    dst = np.asarray(edge_index[1], np.int64)

    # Degree-balanced node relabeling: deal nodes (sorted by in-degree)
    # round-robin across the (core, window) slots so every window sees a
    # near-equal edge count -> minimal static gather padding.
    deg_in = np.bincount(dst, minlength=cfg0.n)
    order_nodes = np.argsort(-deg_in, kind="stable")
    caps = np.full(NC * NWIN, WIN, np.int64)
    caps[NWIN - 1::NWIN] = S - (NWIN - 1) * WIN
    perm = np.empty(cfg0.n, np.int64)
    taken = 0
    for rnd in range(int(caps.max())):
        live = np.nonzero(caps > rnd)[0]
        k = min(len(live), cfg0.n - taken)
        nodes = order_nodes[taken:taken + k]
        taken += k
        lv = live[:k]
        perm[nodes] = (lv // NWIN) * S + (lv % NWIN) * WIN + rnd
        if taken >= cfg0.n:
            break
    assert taken == cfg0.n
    src = perm[src]
    dst = perm[dst]

    trow = (src // S) * SPAD + (src % S)
    lo = trow < HALF
    core = dst // S
    win = (dst % S) // WIN
    dloc = (dst % S) - win * WIN

    grp = (core * NWIN + win) * 2 + (1 - lo.astype(np.int64))
    ngrp = NC * NWIN * 2
    cnt = np.bincount(grp, minlength=ngrp).reshape(NC, NWIN, 2)
    s_low = max(128, int(np.ceil(cnt[:, :, 0].max() / 128) * 128))
    s_high = max(128, int(np.ceil(cnt[:, :, 1].max() / 128) * 128))
    C = (s_low + s_high) // 128

    order = np.argsort(grp, kind="stable")
    grp_s = grp[order]
    starts = np.zeros(ngrp + 1, np.int64)
    np.cumsum(np.bincount(grp_s, minlength=ngrp), out=starts[1:])
    within = np.arange(len(grp_s)) - starts[grp_s]

    glo = grp_s % 2 == 0
    gc = grp_s // (2 * NWIN)
    gw = (grp_s // 2) % NWIN

    idx_low = np.zeros((NC, NWIN, s_low), np.int16)
    idx_high = np.zeros((NC, NWIN, s_high), np.int16)
    dstloc = np.full((NC, NWIN, C * 128), -1.0, np.float16)

    tr_s, dl_s = trow[order], dloc[order]
    m = glo
    idx_low[gc[m], gw[m], within[m]] = tr_s[m].astype(np.int16)
    dstloc[gc[m], gw[m], within[m]] = dl_s[m].astype(np.float16)
    m = ~glo
    idx_high[gc[m], gw[m], within[m]] = (tr_s[m] - HALF).astype(np.int16)
    dstloc[gc[m], gw[m], s_low + within[m]] = dl_s[m].astype(np.float16)

    idxlo_img = np.tile(_wrap16(idx_low).transpose(0, 2, 1, 3)
                        .reshape(NC, 16, NWIN * s_low // 16), (1, 8, 1))
    idxhi_img = np.tile(_wrap16(idx_high).transpose(0, 2, 1, 3)
                        .reshape(NC, 16, NWIN * s_high // 16), (1, 8, 1))
    dst_img = dstloc.reshape(NC, NWIN, C, 128).transpose(0, 3, 1, 2) \
                    .reshape(NC, 128, NWIN * C)

    # ---- pairs: 4 buckets by (src-half, tgt-half)
    PC = cfg0.p // NC
    sn = perm[np.asarray(source_nodes, np.int64)].reshape(NC, PC)
    tn = perm[np.asarray(target_nodes, np.int64)].reshape(NC, PC)
    srow = (sn // S) * SPAD + (sn % S)
    trow_p = (tn // S) * SPAD + (tn % S)
    bucket = (srow >= HALF).astype(np.int64) * 2 + (trow_p >= HALF).astype(np.int64)
    bsz = []
    for b in range(4):
        bsz.append(max(128, int(np.ceil((bucket == b).sum(1).max() / 128) * 128)))
    BP = sum(bsz)
    boff = np.concatenate([[0], np.cumsum(bsz)])

    pidx = np.zeros((NC, 2, BP), np.int16)
    ppos = np.zeros((NC, PC), np.int64)
    for c in range(NC):
        o = np.argsort(bucket[c], kind="stable")
        bc = bucket[c][o]
        within = np.arange(PC) - np.searchsorted(bc, bc)
        pos = boff[bc] + within
        sr, tr = srow[c][o], trow_p[c][o]
        pidx[c, 0, pos] = np.where(sr < HALF, sr, sr - HALF).astype(np.int16)
        pidx[c, 1, pos] = np.where(tr < HALF, tr, tr - HALF).astype(np.int16)
        ppos[c, o] = pos
    pidx_img = np.stack([
        np.tile(np.concatenate([_wrap16(pidx[c, 0]), _wrap16(pidx[c, 1])],
                               axis=1), (8, 1))
        for c in range(NC)])

    cfg = dataclasses.replace(cfg0, s_low=s_low, s_high=s_high, bsz=tuple(bsz))
    meta = dict(idxlo=idxlo_img, idxhi=idxhi_img, dstloc=dst_img,
                pidx=pidx_img, ppos=ppos, boff=boff, perm=perm)
    return cfg, meta


def host_inputs(cfg: Cfg, meta, inputs):
    NC, S = cfg.ncores, cfg.s_nodes
    perm = meta["perm"]
    dst = perm[np.asarray(inputs["edge_index"][1], np.int64)]
    deg = (np.bincount(dst, minlength=cfg.n) + 1).astype(np.int32)
    degp = np.ones((NC, cfg.spad), np.int32)
    degp[:, :S] = deg.reshape(NC, S)
    deg_img = degp.reshape(NC, cfg.nwin, 128).transpose(0, 2, 1)  # [NC,128,NWIN]

    iota = np.tile(np.arange(128, dtype=np.float16)[None, :], (128, cfg.C))
    W1 = np.asarray(inputs["W1"], np.float32)
    PC = cfg.p // NC
    tf = np.asarray(inputs["time_feats"], np.float32).reshape(NC, PC, cfg.tdim)

    common = dict(
        iota=iota,
        ident16=np.eye(128, dtype=np.float16),
        ident32=np.eye(128, dtype=np.float32),
        W_emb=np.asarray(inputs["W_emb"], np.float32),
        Ws=np.asarray(inputs["Ws"], np.float32),
        bemb_t=np.asarray(inputs["b_emb"], np.float32).reshape(cfg.h, 1),
        bs_t=np.ascontiguousarray(np.asarray(inputs["bs"], np.float32).T),
        g_t=np.ascontiguousarray(np.asarray(inputs["gammas"], np.float32).T),
        be_t=np.ascontiguousarray(np.asarray(inputs["betas"], np.float32).T),
        W1a=np.ascontiguousarray(W1[:cfg.h]),
        W1b=np.ascontiguousarray(W1[cfg.h:2 * cfg.h]),
        W1c=np.ascontiguousarray(W1[2 * cfg.h:]),
        b1_t=np.asarray(inputs["b1"], np.float32).reshape(cfg.h, 1),
        W2=np.asarray(inputs["W2"], np.float32),
        b2_t=np.asarray(inputs["b2"], np.float32).reshape(cfg.h2, 1),
        W3=np.asarray(inputs["W3"], np.float32),
        b3_t=np.asarray(inputs["b3"], np.float32).reshape(1, 1),
    )

    x_old = np.asarray(inputs["x"], np.float32)
    x = np.empty_like(x_old)
    x[perm] = x_old
    in_maps = []
    for c in range(NC):
        tfe = np.zeros((cfg.tdim, cfg.bp), np.float32)
        tfe[:, meta["ppos"][c]] = tf[c].T
        m = dict(common)
        m.update(
            xs=np.ascontiguousarray(x[c * S:(c + 1) * S]),
            deg=np.ascontiguousarray(deg_img[c]),
            idxlo=meta["idxlo"][c], idxhi=meta["idxhi"][c],
            dstloc=meta["dstloc"][c], pidx=meta["pidx"][c],
            tfe=tfe,
        )
        in_maps.append(m)
    return in_maps


def assemble_output(cfg: Cfg, meta, results):
    NC, PC = cfg.ncores, cfg.p // cfg.ncores
    y = np.zeros(cfg.p, np.float32)
    for c in range(NC):
        y[c * PC:(c + 1) * PC] = results[c]["yout"][meta["ppos"][c]]
    return y


# ---------------------------------------------------------------------------
# Bass program
# ---------------------------------------------------------------------------

def build_program(cfg: Cfg, debug=False):
    NC = cfg.ncores
    H, H2, S, SPAD, NWIN, C, CL = (cfg.h, cfg.h2, cfg.s_nodes, cfg.spad,
                                   cfg.nwin, cfg.C, cfg.CL)
    SL, SH, HALF = cfg.s_low, cfg.s_high, cfg.half
    BP = cfg.bp
    RG = [list(range(NC))]

    nc = bacc.Bacc("TRN2", target_bir_lowering=False, debug=debug,
                   num_devices=NC, num_swdge_queues=4)
    qctr = [0]

    def next_q():
        q = qctr[0] % 4
        qctr[0] += 1
        return q

    def param(name, shape, dt_):
        return nc.dram_tensor(name, list(shape), dt_, kind="ExternalInput")

    xs = param("xs", (S, cfg.f_in), F32)
    deg = param("deg", (128, NWIN), I32)
    idxlo = param("idxlo", (128, NWIN * SL // 16), I16)
    idxhi = param("idxhi", (128, NWIN * SH // 16), I16)
    dstloc = param("dstloc", (128, NWIN * C), F16)
    iota = param("iota", (128, C * 128), F16)
    ident16 = param("ident16", (128, 128), F16)
    ident32 = param("ident32", (128, 128), F32)
    W_emb = param("W_emb", (cfg.f_in, H), F32)
    Ws = param("Ws", (cfg.L, H, H), F32)
    bemb_t = param("bemb_t", (H, 1), F32)
    bs_t = param("bs_t", (H, cfg.L), F32)
    g_t = param("g_t", (H, cfg.L), F32)
    be_t = param("be_t", (H, cfg.L), F32)
    W1a = param("W1a", (H, H), F32)
    W1b = param("W1b", (H, H), F32)
    W1c = param("W1c", (cfg.tdim, H), F32)
    b1_t = param("b1_t", (H, 1), F32)
    W2 = param("W2", (H, H2), F32)
    b2_t = param("b2_t", (H2, 1), F32)
    W3 = param("W3", (H2, 1), F32)
    b3_t = param("b3_t", (1, 1), F32)
    pidx = param("pidx", (128, 2 * BP // 16), I16)
    tfe = param("tfe", (cfg.tdim, BP), F32)

    yout = nc.dram_tensor("yout", [BP], F32, kind="ExternalOutput")

    T = nc.dram_tensor("Tbl", [cfg.tpad, H], F16, addr_space="Shared")
    agin = nc.dram_tensor("agin", [SPAD, H], F16)
    stin = nc.dram_tensor("stin", [H, 2], F32)
    stout = nc.dram_tensor("stout", [H, 2], F32, addr_space="Shared")
    wgin = nc.dram_tensor("wgin", [1024], F16)
    wgout = nc.dram_tensor("wgout", [1024 * NC], F16, addr_space="Shared")

    with tile.TileContext(nc) as tc, \
         tc.tile_pool(name="const", bufs=1) as cp, \
         tc.tile_pool(name="sm", bufs=2) as smp, \
         tc.tile_pool(name="paggr", bufs=3, space="PSUM") as paggr, \
         tc.tile_pool(name="ptr", bufs=2, space="PSUM") as ptr, \
         tc.tile_pool(name="pz", bufs=2, space="PSUM") as pz:

        def load(p, shape, dt_, src, tag):
            t = p.tile(list(shape), dt_, tag=tag)
            nc.sync.dma_start(out=t[:], in_=src)
            return t

        iota_t = load(cp, (128, C * 128), F16, iota.ap(), "iota")
        id16 = load(cp, (128, 128), F16, ident16.ap(), "id16")
        id32 = load(cp, (128, 128), F32, ident32.ap(), "id32")
        ilo = load(cp, (128, NWIN * SL // 16), I16, idxlo.ap(), "ilo")
        ihi = load(cp, (128, NWIN * SH // 16), I16, idxhi.ap(), "ihi")
        dl_t = load(cp, (128, NWIN * C), F16, dstloc.ap(), "dlt")
        Wemb_t = load(cp, (cfg.f_in, H), F32, W_emb.ap(), "wemb")
        Wl_t = [load(cp, (H, H), F32, Ws.ap()[i], tag=f"Wl{i}")
                for i in range(cfg.L)]
        bemb = load(cp, (H, 1), F32, bemb_t.ap(), "bemb")
        bsl = load(cp, (H, cfg.L), F32, bs_t.ap(), "bsl")
        gl = load(cp, (H, cfg.L), F32, g_t.ap(), "gl")
        bel = load(cp, (H, cfg.L), F32, be_t.ap(), "bel")
        W1a_t = load(cp, (H, H), F32, W1a.ap(), "w1a")
        W1b_t = load(cp, (H, H), F32, W1b.ap(), "w1b")
        W1c_t = load(cp, (cfg.tdim, H), F32, W1c.ap(), "w1c")
        b1 = load(cp, (H, 1), F32, b1_t.ap(), "b1")
        W2_t = load(cp, (H, H2), F32, W2.ap(), "w2")
        b2 = load(cp, (H2, 1), F32, b2_t.ap(), "b2")
        W3_t = load(cp, (H2, 1), F32, W3.ap(), "w3")
        b3 = load(cp, (1, 1), F32, b3_t.ap(), "b3")

        def cast16(src, shape, tag):
            t = cp.tile(list(shape), F16, tag=tag)
            nc.vector.tensor_copy(out=t[:], in_=src[:])
            return t
        W1a16 = cast16(W1a_t, (H, H), "w1a16")
        W1b16 = cast16(W1b_t, (H, H), "w1b16")
        W1c16 = cast16(W1c_t, (cfg.tdim, H), "w1c16")
        W216 = cast16(W2_t, (H, H2), "w216")
        W316 = cast16(W3_t, (H2, 1), "w316")

        epst = cp.tile([128, 1], F32, tag="eps")
        nc.vector.memset(epst[:], float(cfg.eps))

        # ---- dinv ----------------------------------------------------------
        deg_t = load(cp, (128, NWIN), I32, deg.ap(), "deg")
        degf = cp.tile([128, NWIN], F32, tag="degf")
        nc.vector.tensor_copy(out=degf[:], in_=deg_t[:])
        sq = cp.tile([128, NWIN], F32, tag="sqdeg")
        nc.scalar.activation(out=sq[:], in_=degf[:], func=AF.Sqrt)
        dinv_nm = cp.tile([128, NWIN], F32, tag="dinvnm")
        nc.vector.reciprocal(out=dinv_nm[:], in_=sq[:])
        dinvd = nc.dram_tensor("dinvd", [SPAD], F32)
        nc.sync.dma_start(out=dinvd.ap().rearrange("(w p) -> p w", p=128),
                          in_=dinv_nm[:])

        with tc.tile_pool(name="big", bufs=1) as bigp:

            dinvrep = bigp.tile([128, SPAD], F32)
            nc.sync.dma_start(
                out=dinvrep[:],
                in_=dinvd.ap().unsqueeze(0).broadcast_to([128, SPAD]))

            stage = bigp.tile([128, SPAD], F16)
            nm = bigp.tile([128, SPAD], F16)
            vs = bigp.tile([128, SPAD], F32)
            hpre = bigp.tile([128, SPAD], F32)
            if SPAD > S:
                nc.vector.memset(stage[:, S:SPAD], 0.0)
                nc.vector.memset(vs[:, S:SPAD], 0.0)

            def zchunks():
                k = 0
                while k < SPAD:
                    yield k, min(512, SPAD - k)
                    k += 512

            # ---- embed -----------------------------------------------------
            with tc.tile_pool(name="emb", bufs=1) as embp:
                xload = embp.tile([128, NWIN * cfg.f_in], F32)
                nc.vector.memset(xload[:], 0.0)
                nfull = S // 128
                x3 = xload[:].rearrange("p (w f) -> p w f", f=cfg.f_in)
                nc.sync.dma_start(
                    out=x3[:, :nfull, :],
                    in_=xs.ap()[:nfull * 128, :]
                        .rearrange("(w p) f -> p w f", p=128))
                rem = S - nfull * 128
                if rem:
                    nc.sync.dma_start(
                        out=x3[:rem, nfull, :],
                        in_=xs.ap()[nfull * 128:, :]
                            .rearrange("(w p) f -> p w f", p=rem))
                xT = embp.tile([cfg.f_in, SPAD], F32)
                for w in range(NWIN):
                    pt = ptr.tile([cfg.f_in, 128], F32, tag="tr")
                    nc.tensor.transpose(
                        pt[:], xload[:, w * cfg.f_in:(w + 1) * cfg.f_in],
                        id32[:])
                    nc.vector.tensor_copy(out=xT[:, w * 128:(w + 1) * 128],
                                          in_=pt[:])
                for k, cw in zchunks():
                    pzt = pz.tile([128, 512], F32, tag="z")
                    nc.tensor.matmul(pzt[:, :cw], lhsT=Wemb_t[:],
                                     rhs=xT[:, k:k + cw],
                                     start=True, stop=True)
                    nc.scalar.activation(out=hpre[:, k:k + cw],
                                         in_=pzt[:, :cw],
                                         func=AF.Relu, bias=bemb[:])
                nc.vector.tensor_tensor(out=stage[:, :S], in0=hpre[:, :S],
                                        in1=dinvrep[:, :S], op=ALU.mult)

            def stage_to_T():
                for w in range(NWIN):
                    pt2 = ptr.tile([128, 128], F16, tag="tr")
                    nc.tensor.transpose(pt2[:],
                                        stage[:, w * 128:(w + 1) * 128],
                                        id16[:])
                    nc.vector.tensor_copy(out=nm[:, w * 128:(w + 1) * 128],
                                          in_=pt2[:])
                nc.sync.dma_start(
                    out=agin.ap().rearrange("(w p) f -> p w f", p=128),
                    in_=nm[:].rearrange("p (w f) -> p w f", f=H))
                nc.gpsimd.collective_compute(
                    "AllGather", ALU.bypass, replica_groups=RG,
                    ins=[agin.ap().opt()], outs=[T.ap().opt()])

            stage_to_T()

            # ---- layers ----------------------------------------------------
            T_low = T.ap()
            T_high = T.ap()[HALF:, :]
            iota3 = iota_t[:].rearrange("p (c e) -> p c e", e=128)

            msgp = tc.alloc_tile_pool(name="msg", bufs=3)
            ohp = tc.alloc_tile_pool(name="oh", bufs=3)
            NCH = (SPAD + 511) // 512
            for i in range(cfg.L):
                sum_p = smp.tile([128, NCH], F32, tag="sump")
                sumsq_p = smp.tile([128, NCH], F32, tag="sumsqp")
                for w in range(NWIN):
                    ml = msgp.tile([128, CL, 128], F16, tag="mlo")
                    mh = msgp.tile([128, C - CL, 128], F16, tag="mhi")
                    if i == 0 and w < 3:
                        nc.vector.memset(ml[:], 0.0)
                        nc.vector.memset(mh[:], 0.0)
                    MAXG = 1024
                    for g0 in range(0, SL, MAXG):
                        gn = min(MAXG, SL - g0)
                        nc.gpsimd.dma_gather(
                            ml[:, g0 // 128:(g0 + gn) // 128, :], T_low,
                            ilo[:, (w * SL + g0) // 16:(w * SL + g0 + gn) // 16],
                            gn, gn, H, queue_num=next_q())
                    for g0 in range(0, SH, MAXG):
                        gn = min(MAXG, SH - g0)
                        nc.gpsimd.dma_gather(
                            mh[:, g0 // 128:(g0 + gn) // 128, :], T_high,
                            ihi[:, (w * SH + g0) // 16:(w * SH + g0 + gn) // 16],
                            gn, gn, H, queue_num=next_q())
                    oh = ohp.tile([128, C, 128], F16, tag="oh")
                    nc.vector.tensor_tensor(
                        out=oh[:], in0=iota3,
                        in1=dl_t[:, w * C:(w + 1) * C].unsqueeze(2)
                            .broadcast_to([128, C, 128]),
                        op=ALU.is_equal)
                    pw = paggr.tile([128, 128], F32, tag="agg")
                    for c in range(C):
                        lhsT = ml[:, c, :] if c < CL else mh[:, c - CL, :]
                        nc.tensor.matmul(pw[:], lhsT=lhsT, rhs=oh[:, c, :],
                                         start=(c == 0), stop=(c == C - 1))
                    cw = min(128, S - w * 128)
                    sl = slice(w * 128, w * 128 + cw)
                    nc.vector.tensor_tensor(out=vs[:, sl], in0=pw[:, :cw],
                                            in1=stage[:, sl], op=ALU.add)
                    nc.vector.tensor_tensor(out=vs[:, sl], in0=vs[:, sl],
                                            in1=dinvrep[:, sl], op=ALU.mult)
                    # dense z = W @ vs + stats for the finished 512-chunk,
                    # interleaved so it hides under the next window's gathers
                    if (w + 1) % 4 == 0 or w == NWIN - 1:
                        j = w // 4
                        k = j * 512
                        zw = min(512, SPAD - k)
                        sw = min(zw, max(0, S - k))
                        pzt = pz.tile([128, 512], F32, tag="z")
                        nc.tensor.matmul(pzt[:, :zw],
                                         lhsT=Wl_t[i][:],
                                         rhs=vs[:, k:k + zw],
                                         start=True, stop=True)
                        nc.scalar.activation(out=hpre[:, k:k + zw],
                                             in_=pzt[:, :zw],
                                             func=AF.Identity,
                                             bias=bsl[:, i:i + 1])
                        nc.vector.tensor_reduce(sum_p[:, j:j + 1],
                                                hpre[:, k:k + sw],
                                                axis=mybir.AxisListType.X,
                                                op=ALU.add)
                        nc.vector.scalar_tensor_tensor(
                            out=vs[:, k:k + sw], in0=hpre[:, k:k + sw],
                            scalar=1.0, in1=hpre[:, k:k + sw],
                            op0=ALU.mult, op1=ALU.mult,
                            accum_out=sumsq_p[:, j:j + 1])

                # global BatchNorm stats
                sumt = smp.tile([128, 1], F32, tag="sum")
                sumsq = smp.tile([128, 1], F32, tag="sumsq")
                nc.vector.tensor_reduce(sumt[:], sum_p[:],
                                        axis=mybir.AxisListType.X, op=ALU.add)
                nc.vector.tensor_reduce(sumsq[:], sumsq_p[:],
                                        axis=mybir.AxisListType.X, op=ALU.add)
                stats = smp.tile([128, 2], F32, tag="stats")
                nc.vector.tensor_copy(out=stats[:, 0:1], in_=sumt[:])
                nc.vector.tensor_copy(out=stats[:, 1:2], in_=sumsq[:])
                nc.sync.dma_start(out=stin.ap(), in_=stats[:])
                nc.gpsimd.collective_compute(
                    "AllReduce", ALU.add, replica_groups=RG,
                    ins=[stin.ap().opt()], outs=[stout.ap().opt()])
                stg = smp.tile([128, 2], F32, tag="stg")
                nc.sync.dma_start(out=stg[:], in_=stout.ap())

                mu = smp.tile([128, 1], F32, tag="mu")
                nc.vector.tensor_scalar(out=mu[:], in0=stg[:, 0:1],
                                        scalar1=1.0 / cfg.n, scalar2=None,
                                        op0=ALU.mult)
                musq = smp.tile([128, 1], F32, tag="musq")
                nc.vector.tensor_tensor(out=musq[:], in0=mu[:], in1=mu[:],
                                        op=ALU.mult)
                var = smp.tile([128, 1], F32, tag="var")
                nc.vector.scalar_tensor_tensor(out=var[:], in0=stg[:, 1:2],
                                               scalar=1.0 / cfg.n,
                                               in1=musq[:],
                                               op0=ALU.mult, op1=ALU.subtract)
                std = smp.tile([128, 1], F32, tag="std")
                nc.scalar.activation(out=std[:], in_=var[:], func=AF.Sqrt,
                                     bias=epst[:])
                rstd = smp.tile([128, 1], F32, tag="rstd")
                nc.vector.reciprocal(out=rstd[:], in_=std[:])
                scl = smp.tile([128, 1], F32, tag="scl")
                nc.vector.tensor_tensor(out=scl[:], in0=rstd[:],
                                        in1=gl[:, i:i + 1], op=ALU.mult)
                msc = smp.tile([128, 1], F32, tag="msc")
                nc.vector.tensor_tensor(out=msc[:], in0=mu[:], in1=scl[:],
                                        op=ALU.mult)
                shift = smp.tile([128, 1], F32, tag="shift")
                nc.vector.tensor_tensor(out=shift[:], in0=bel[:, i:i + 1],
                                        in1=msc[:], op=ALU.subtract)

                if i < cfg.L - 1:
                    nc.scalar.activation(out=vs[:, :S], in_=hpre[:, :S],
                                         func=AF.Relu, bias=shift[:],
                                         scale=scl[:])
                    nc.vector.tensor_tensor(out=stage[:, :S], in0=vs[:, :S],
                                            in1=dinvrep[:, :S], op=ALU.mult)
                else:
                    nc.scalar.activation(out=stage[:, :S], in_=hpre[:, :S],
                                         func=AF.Identity, bias=shift[:],
                                         scale=scl[:])
                stage_to_T()
            ohp.release()
            msgp.release()

        # ---- pair MLP (chunked) -------------------------------------------
        with tc.tile_pool(name="pair", bufs=2) as pp, \
             tc.tile_pool(name="pairc", bufs=1) as ppc:
            pidx_t = load(ppc, (128, 2 * BP // 16), I16, pidx.ap(), "pidx")
            T_low = T.ap()
            T_high = T.ap()[HALF:, :]
            boff = [0]
            for b in cfg.bsz:
                boff.append(boff[-1] + b)
            halves = [(0, 0), (0, 1), (1, 0), (1, 1)]
            PCH = cfg.pchunk
            for b in range(4):
                for k0 in range(boff[b], boff[b + 1], PCH):
                    cw = min(PCH, boff[b + 1] - k0)
                    nb = cw // 128
                    gs = pp.tile([128, PCH // 128, 128], F16, tag="gs")
                    gt = pp.tile([128, PCH // 128, 128], F16, tag="gt")
                    MAXG = 1024
                    for g0 in range(0, cw, MAXG):
                        gn = min(MAXG, cw - g0)
                        nc.gpsimd.dma_gather(
                            gs[:, g0 // 128:(g0 + gn) // 128, :],
                            T_high if halves[b][0] else T_low,
                            pidx_t[:, (k0 + g0) // 16:(k0 + g0 + gn) // 16],
                            gn, gn, H, queue_num=next_q())
                        nc.gpsimd.dma_gather(
                            gt[:, g0 // 128:(g0 + gn) // 128, :],
                            T_high if halves[b][1] else T_low,
                            pidx_t[:, (BP + k0 + g0) // 16:(BP + k0 + g0 + gn) // 16],
                            gn, gn, H, queue_num=next_q())
                    hsT = pp.tile([128, PCH], F16, tag="hsT")
                    htT = pp.tile([128, PCH], F16, tag="htT")
                    for t in range(nb):
                        for g, dstT in ((gs, hsT), (gt, htT)):
                            pt2 = ptr.tile([128, 128], F16, tag="tr")
                            nc.tensor.transpose(pt2[:], g[:, t, :], id16[:])
                            nc.vector.tensor_copy(
                                out=dstT[:, t * 128:(t + 1) * 128], in_=pt2[:])
                    tfc = pp.tile([cfg.tdim, PCH], F32, tag="tfc")
                    nc.sync.dma_start(out=tfc[:, :cw],
                                      in_=tfe.ap()[:, k0:k0 + cw])
                    tfc16 = pp.tile([cfg.tdim, PCH], F16, tag="tfc16")
                    nc.vector.tensor_copy(out=tfc16[:, :cw], in_=tfc[:, :cw])
                    y1T = pp.tile([128, PCH], F16, tag="y1T")
                    y2T = pp.tile([H2, PCH], F16, tag="y2T")
                    ysb = pp.tile([1, PCH], F32, tag="ysb")
                    for kk in range(0, cw, 512):
                        cc = min(512, cw - kk)
                        p1 = pz.tile([128, 512], F32, tag="z")
                        nc.tensor.matmul(p1[:, :cc], lhsT=W1a16[:],
                                         rhs=hsT[:, kk:kk + cc],
                                         start=True, stop=False)
                        nc.tensor.matmul(p1[:, :cc], lhsT=W1b16[:],
                                         rhs=htT[:, kk:kk + cc],
                                         start=False, stop=False)
                        nc.tensor.matmul(p1[:, :cc], lhsT=W1c16[:],
                                         rhs=tfc16[:, kk:kk + cc],
                                         start=False, stop=True)
                        nc.scalar.activation(out=y1T[:, kk:kk + cc],
                                             in_=p1[:, :cc],
                                             func=AF.Relu, bias=b1[:])
                        p2 = pz.tile([H2, 512], F32, tag="z")
                        nc.tensor.matmul(p2[:, :cc], lhsT=W216[:],
                                         rhs=y1T[:, kk:kk + cc],
                                         start=True, stop=True)
                        nc.scalar.activation(out=y2T[:, kk:kk + cc],
                                             in_=p2[:, :cc],
                                             func=AF.Relu, bias=b2[:])
                        p3 = pz.tile([1, 512], F32, tag="z")
                        nc.tensor.matmul(p3[:, :cc], lhsT=W316[:],
                                         rhs=y2T[:, kk:kk + cc],
                                         start=True, stop=True)
                        nc.scalar.activation(out=ysb[:, kk:kk + cc],
                                             in_=p3[:, :cc],
                                             func=AF.Identity, bias=b3[:])
                    nc.sync.dma_start(
                        out=yout.ap()[k0:k0 + cw].unsqueeze(0),
                        in_=ysb[:, :cw])

    nc.compile()
    return nc


# ---------------------------------------------------------------------------
# Entry point
# ---------------------------------------------------------------------------

_CACHE = {}


def kernel(**inputs):
    from concourse.bass_utils import run_bass_kernel_spmd

    cfg0 = Cfg()
    cfg, meta = host_prep(cfg0, inputs["edge_index"],
                          inputs["source_nodes"], inputs["target_nodes"])
    in_maps = host_inputs(cfg, meta, inputs)
    nc = _CACHE.get(cfg)
    if nc is None:
        nc = build_program(cfg, debug=False)
        _CACHE[cfg] = nc
    res = run_bass_kernel_spmd(nc, in_maps, list(range(cfg.ncores)))
    return assemble_output(cfg, meta, res.results)

